# revision 47
# baseline (speedup 1.0000x reference)
"""BiDense (binary dense) kernel for Trainium2, column-parallel over 8 NeuronCores.

Math (mirrors the reference exactly):
    bk[f] = max_d |kernel[d, f]| + f32_eps          (per-output-feature bound)
    bx[t] = max_d |x[t, d]|      + f32_eps          (per-token bound)
    kq = sign*(kernel) * 0.5 * bk[f]                (sign* maps 0 -> +1)
    xq = sign*(x)      * 0.5 * bx[t]
    y[t, f] = sum_d xq kq + bias[f]
            = 0.25 * bx[t] * bk[f] * (Sx @ Sk)[t, f] + bias[f]

Sx/Sk are +-1 matrices, so the GEMM runs exactly in fp8/bf16 (products are
+-1, accumulation of <=4096 integers is exact in fp32 PSUM).  The reference's
fp32 accumulation reduces to fl(0.25*bx*bk) * integer as well, so results
match to ~1e-7.

Sharding: column-parallel (tensor-parallel over features).  Each core gets
the full x and a 1/8 slice of kernel/bias along f; outputs concat along f.
"""

import numpy as np
from contextlib import ExitStack

import concourse.bass as bass
import concourse.bass_isa as bass_isa
import concourse.mybir as mybir
import concourse.tile as tile
from concourse import bacc, bass_utils
from concourse.masks import make_identity

P = 128
N_CORES = 8
F32_EPS = float(np.finfo(np.float32).eps)
SIGN_BIAS = 1e-30  # sign(v + tiny): maps v==0 to +1, never flips a real value

FP32 = mybir.dt.float32
ALU = mybir.AluOpType
AX = mybir.AxisListType


def build_nc(T, D, F, mm_dt=mybir.dt.float8e4, double_row=True, has_bias=False,
             NF=512, TG=4, tr_mode="sign8", PRE=7):
    """Build the per-core Bass program.

    T: tokens (rows of x) handled by this core (full T here)
    D: contraction dim
    F: features handled by this core (the shard)
    tr_mode: "f32"   - PE-transpose raw fp32 x, fuse sign into psum evacuation
             "sign8" - sign first (ACT) into mm_dt, PE-transpose the 1-byte
                       signs (1-pass instead of fp32's LOW_HIGH 2-pass)
    PRE: token blocks transposed ahead (overlaps the kernel-shard DMA preamble)
    """
    assert T % P == 0 and D % P == 0 and F % NF == 0 and NF % P == 0
    KT = D // P            # contraction tiles
    TB = T // P            # token blocks
    FC = F // NF           # psum chunks along f
    NG = KT // TG          # transpose groups per token block
    assert KT % TG == 0
    PRE = min(PRE, TB)
    if double_row:
        assert mm_dt in (mybir.dt.float8e4, mybir.dt.float8e5) and KT % 2 == 0

    nc = bacc.Bacc(trn_type="TRN2")
    x_d = nc.dram_tensor("x_in", [T, D], FP32, kind="ExternalInput")
    k_d = nc.dram_tensor("k_in", [D, F], FP32, kind="ExternalInput")
    b_d = None
    if has_bias:
        b_d = nc.dram_tensor("b_in", [F], FP32, kind="ExternalInput")
    y_d = nc.dram_tensor("y_out", [T, F], FP32, kind="ExternalOutput")

    with ExitStack() as ctx:
        tc = ctx.enter_context(tile.TileContext(nc))
        const = ctx.enter_context(tc.tile_pool(name="const", bufs=1))
        skp = ctx.enter_context(tc.tile_pool(name="sk", bufs=1))
        tpps = ctx.enter_context(tc.tile_pool(name="tpps", bufs=2, space="PSUM"))
        mmps = ctx.enter_context(tc.tile_pool(name="mmps", bufs=FC + 2, space="PSUM"))
        xp = ctx.enter_context(tc.tile_pool(name="xp", bufs=2))
        sxtp = ctx.enter_context(tc.tile_pool(name="sxtp", bufs=PRE + 2))
        outp = ctx.enter_context(tc.tile_pool(name="outp", bufs=4))
        bxp = ctx.enter_context(tc.tile_pool(name="bxp", bufs=PRE + 28))
        dramp = ctx.enter_context(tc.tile_pool(name="dram", bufs=1,
                                               space="DRAM"))
        sxnp = (ctx.enter_context(tc.tile_pool(name="sxnp", bufs=2))
                if tr_mode == "sign8" else None)

        tr_dt = mm_dt if tr_mode == "sign8" else FP32
        ident = const.tile([P, P], tr_dt)
        make_identity(nc, ident)
        sbias = const.tile([P, 1], FP32)   # tiny bias so sign(0+eps) = +1
        nc.vector.memset(sbias, SIGN_BIAS)

        sk = skp.tile([P, KT, F], mm_dt)          # Sk signs, [d_lo, d_hi, f]
        bkb = const.tile([P, F], FP32)            # 0.5*(bk+eps), bcast on parts
        biasb = const.tile([P, F], FP32) if has_bias else None
        halfb = const.tile([P, 1], FP32)
        nc.vector.memset(halfb, 0.5)

        mxk = const.tile([P, F], FP32)
        mnk = const.tile([P, F], FP32)

        def emit_k1(kload, kt):
            # one kernel stream: DMA on the (idle) gpsimd queue so x loads on
            # sync are never blocked; ACT computes Sk signs (+-1, 0 -> +1),
            # DVE tracks running max/min for the per-feature bounds.
            ktile = kload.tile([P, F], FP32, tag="kl", name="ktile")
            nc.gpsimd.dma_start(ktile, k_d[kt * P:(kt + 1) * P, :])
            nc.scalar.sign(sk[:, kt, :], ktile, bias=sbias[:])
            if kt == 0:
                nc.vector.tensor_copy(mxk, ktile)
                nc.vector.tensor_copy(mnk, ktile)
            else:
                nc.vector.tensor_tensor(mxk, mxk, ktile, op=ALU.max)
                nc.vector.tensor_tensor(mnk, mnk, ktile, op=ALU.min)

        def emit_bounds_final():
            # bk = max(mx, -mn) reduced across partitions on GPSIMD, already
            # broadcast over partitions with f on the free axis
            nc.vector.scalar_tensor_tensor(
                mxk, mnk, -1.0, mxk, op0=ALU.mult, op1=ALU.max)
            nc.gpsimd.partition_all_reduce(bkb, mxk, channels=P,
                                           reduce_op=bass_isa.ReduceOp.max)
            nc.vector.tensor_scalar(bkb, bkb, F32_EPS, 0.25,
                                    op0=ALU.add, op1=ALU.mult)
            if has_bias:
                bsrc = b_d[:]
                bbcast = bass.AP(tensor=bsrc.tensor, offset=bsrc.offset,
                                 ap=[[0, P]] + [list(pair) for pair in bsrc.ap])
                nc.sync.dma_start(biasb, bbcast)

        # ---- main loop over token blocks ----------------------------------
        # Early blocks write raw matmul counts to a DRAM scratch (their
        # epilogue is a plain ACT psum-copy with no dependency on bounds);
        # they are re-read, scaled and written to y once bkb is ready.
        kstep = 2 if double_row else 1
        pm = mybir.MatmulPerfMode.DoubleRow if double_row else None
        bx_tiles = {}
        src_tiles = {}   # per-block transpose source (x fp32 or fp8 signs)
        sxt_tiles = {}

        def emit_xload(j):
            x_t = xp.tile([P, D], FP32, tag="x", name="x_t")
            nc.sync.dma_start(x_t, x_d[j * P:(j + 1) * P, :])
            bxq = bxp.tile([P, 1], FP32, tag="bx", name="bxq")
            nc.vector.tensor_reduce(bxq, x_t, axis=AX.X, op=ALU.max,
                                    apply_absolute_value=True)
            nc.vector.tensor_scalar_add(bxq, bxq, F32_EPS)
            bx_tiles[j] = bxq
            if tr_mode == "sign8":
                sxn = sxnp.tile([P, D], mm_dt, tag="sxn", name="sxn")
                nc.scalar.sign(sxn, x_t, bias=sbias[:])
                src_tiles[j] = sxn
            else:
                src_tiles[j] = x_t
            sxt_tiles[j] = sxtp.tile([P, KT, P], mm_dt, tag="sxt", name="sxt")

        def emit_tgroup(j, g):
            # PE-transpose TG [128,128] chunks into one psum bank, then ACT
            # evacuates into sxt (computing sign for the fp32 path).
            src = src_tiles[j]
            if tr_mode == "sign8":
                # fp8 transpose mode requires output element step of 2
                tp = tpps.tile([P, TG * P, 2], tr_dt, tag="tp", name="tp")
                tpw = tp[:, :, 0]
            else:
                tp = tpps.tile([P, TG * P], tr_dt, tag="tp", name="tp")
                tpw = tp[:]
            for u in range(TG):
                kt = g * TG + u
                nc.tensor.transpose(tpw[:, u * P:(u + 1) * P],
                                    src[:, kt * P:(kt + 1) * P], ident)
            dst = sxt_tiles[j][:, g * TG:(g + 1) * TG, :]
            tpv = tpw.rearrange("p (u t) -> p u t", u=TG)
            if tr_mode == "sign8":
                nc.scalar.copy(dst, tpv)
            else:
                nc.scalar.sign(dst, tpv, bias=sbias[:])

        # merged preamble: transpose-ahead interleaved with kernel stream-1
        # slices, so PE/ACT start at once and sk tiles arrive progressively
        k1_per_j = (KT + PRE - 1) // PRE
        k1_next = 0
        with tc.tile_pool(name="kload", bufs=4) as kload:
            for j in range(PRE):
                emit_xload(j)
                for g in range(NG):
                    emit_tgroup(j, g)
                for _ in range(k1_per_j):
                    if k1_next < KT:
                        emit_k1(kload, k1_next)
                        k1_next += 1
            while k1_next < KT:
                emit_k1(kload, k1_next)
                k1_next += 1
            emit_bounds_final()

        for i in range(TB):
            if i + PRE < TB:
                emit_xload(i + PRE)
            sxt = sxt_tiles.pop(i)
            src_tiles.pop(i, None)
            mm_tiles = [mmps.tile([P, NF], FP32, tag="mm", name=f"mm{fc}")
                        for fc in range(FC)]

            for g in range(NG):
                for kt in range(g * TG, (g + 1) * TG, kstep):
                    start = kt == 0
                    stop = kt + kstep >= KT
                    for fc in range(FC):
                        if double_row:
                            nc.tensor.matmul(
                                mm_tiles[fc][:],
                                lhsT=sxt[:, kt:kt + 2, :],
                                rhs=sk[:, kt:kt + 2, fc * NF:(fc + 1) * NF],
                                start=start, stop=stop, perf_mode=pm)
                        else:
                            nc.tensor.matmul(
                                mm_tiles[fc][:],
                                lhsT=sxt[:, kt, :],
                                rhs=sk[:, kt, fc * NF:(fc + 1) * NF],
                                start=start, stop=stop)
                # keep PE's non-HAM-warm transpose stretches short by
                # interleaving the lookahead block's transpose groups
                if i + PRE < TB:
                    emit_tgroup(i + PRE, g)

            bxq = bx_tiles.pop(i)
            for fc in range(FC):
                sl = slice(fc * NF, (fc + 1) * NF)
                out_c = outp.tile([P, NF], FP32, tag="out", name="out_c")
                # y = (psum * bx[t]) * (0.25*bk[f])
                nc.vector.scalar_tensor_tensor(
                    out_c, mm_tiles[fc][:], bxq, bkb[:, sl],
                    op0=ALU.mult, op1=ALU.mult)
                if has_bias:
                    nc.vector.tensor_tensor(out_c, out_c, biasb[:, sl],
                                            op=ALU.add)
                nc.sync.dma_start(y_d[i * P:(i + 1) * P, sl], out_c)

    if not nc.is_finalized():
        nc.finalize()
    return nc


def _run(x2, ksh_list, bias_list, has_bias, mm_dt=mybir.dt.float8e4,
         double_row=True, trace=False, NF=512, tr_mode="sign8", PRE=7):
    """Compile once and run the SPMD program on all 8 cores."""
    T, D = x2.shape
    F = ksh_list[0].shape[1]
    nc = build_nc(T, D, F, mm_dt=mm_dt, double_row=double_row,
                  has_bias=has_bias, NF=NF, tr_mode=tr_mode, PRE=PRE)
    in_maps = []
    for c in range(len(ksh_list)):
        m = {"x_in": x2, "k_in": ksh_list[c]}
        if has_bias:
            m["b_in"] = bias_list[c]
        in_maps.append(m)
    res = bass_utils.run_bass_kernel_spmd(
        nc, in_maps, core_ids=list(range(len(ksh_list))), trace=trace)
    return res


def kernel(x, kernel, bias):
    x = np.ascontiguousarray(np.asarray(x, dtype=np.float32))
    k = np.ascontiguousarray(np.asarray(kernel, dtype=np.float32))
    b = np.ascontiguousarray(np.asarray(bias, dtype=np.float32))
    B, S, D = x.shape
    F = k.shape[1]
    T = B * S
    FS = F // N_CORES
    x2 = np.ascontiguousarray(x.reshape(T, D))
    has_bias = bool(np.any(b))
    ksh = [np.ascontiguousarray(k[:, c * FS:(c + 1) * FS]) for c in range(N_CORES)]
    bsh = [np.ascontiguousarray(b[c * FS:(c + 1) * FS]) for c in range(N_CORES)]
    res = _run(x2, ksh, bsh, has_bias)
    y = np.concatenate([res.results[c]["y_out"] for c in range(N_CORES)], axis=1)
    return np.ascontiguousarray(y.reshape(B, S, F)).astype(np.float32)


# revision 50
# speedup vs baseline: 1.0077x; 1.0077x over previous
"""BiDense (binary dense) kernel for Trainium2, column-parallel over 8 NeuronCores.

Math (mirrors the reference exactly):
    bk[f] = max_d |kernel[d, f]| + f32_eps          (per-output-feature bound)
    bx[t] = max_d |x[t, d]|      + f32_eps          (per-token bound)
    kq = sign*(kernel) * 0.5 * bk[f]                (sign* maps 0 -> +1)
    xq = sign*(x)      * 0.5 * bx[t]
    y[t, f] = sum_d xq kq + bias[f]
            = 0.25 * bx[t] * bk[f] * (Sx @ Sk)[t, f] + bias[f]

Sx/Sk are +-1 matrices, so the GEMM runs exactly in fp8/bf16 (products are
+-1, accumulation of <=4096 integers is exact in fp32 PSUM).  The reference's
fp32 accumulation reduces to fl(0.25*bx*bk) * integer as well, so results
match to ~1e-7.

Sharding: column-parallel (tensor-parallel over features).  Each core gets
the full x and a 1/8 slice of kernel/bias along f; outputs concat along f.
"""

import numpy as np
from contextlib import ExitStack

import concourse.bass as bass
import concourse.bass_isa as bass_isa
import concourse.mybir as mybir
import concourse.tile as tile
from concourse import bacc, bass_utils
from concourse.masks import make_identity

P = 128
N_CORES = 8
F32_EPS = float(np.finfo(np.float32).eps)
SIGN_BIAS = 1e-30  # sign(v + tiny): maps v==0 to +1, never flips a real value

FP32 = mybir.dt.float32
ALU = mybir.AluOpType
AX = mybir.AxisListType


def build_nc(T, D, F, mm_dt=mybir.dt.float8e4, double_row=True, has_bias=False,
             NF=512, TG=4, tr_mode="sign8", PRE=5):
    """Build the per-core Bass program.

    T: tokens (rows of x) handled by this core (full T here)
    D: contraction dim
    F: features handled by this core (the shard)
    tr_mode: "f32"   - PE-transpose raw fp32 x, fuse sign into psum evacuation
             "sign8" - sign first (ACT) into mm_dt, PE-transpose the 1-byte
                       signs (1-pass instead of fp32's LOW_HIGH 2-pass)
    PRE: token blocks transposed ahead (overlaps the kernel-shard DMA preamble)
    """
    assert T % P == 0 and D % P == 0 and F % NF == 0 and NF % P == 0
    KT = D // P            # contraction tiles
    TB = T // P            # token blocks
    FC = F // NF           # psum chunks along f
    NG = KT // TG          # transpose groups per token block
    assert KT % TG == 0
    PRE = min(PRE, TB)
    if double_row:
        assert mm_dt in (mybir.dt.float8e4, mybir.dt.float8e5) and KT % 2 == 0

    nc = bacc.Bacc(trn_type="TRN2")
    x_d = nc.dram_tensor("x_in", [T, D], FP32, kind="ExternalInput")
    k_d = nc.dram_tensor("k_in", [D, F], FP32, kind="ExternalInput")
    b_d = None
    if has_bias:
        b_d = nc.dram_tensor("b_in", [F], FP32, kind="ExternalInput")
    y_d = nc.dram_tensor("y_out", [T, F], FP32, kind="ExternalOutput")

    with ExitStack() as ctx:
        tc = ctx.enter_context(tile.TileContext(nc))
        const = ctx.enter_context(tc.tile_pool(name="const", bufs=1))
        skp = ctx.enter_context(tc.tile_pool(name="sk", bufs=1))
        tpps = ctx.enter_context(tc.tile_pool(name="tpps", bufs=2, space="PSUM"))
        mmps = ctx.enter_context(tc.tile_pool(name="mmps", bufs=FC + 2, space="PSUM"))
        xp = ctx.enter_context(tc.tile_pool(name="xp", bufs=2))
        sxtp = ctx.enter_context(tc.tile_pool(name="sxtp", bufs=PRE + 2))
        outp = ctx.enter_context(tc.tile_pool(name="outp", bufs=4))
        bxp = ctx.enter_context(tc.tile_pool(name="bxp", bufs=PRE + 28))
        dramp = ctx.enter_context(tc.tile_pool(name="dram", bufs=1,
                                               space="DRAM"))
        sxnp = (ctx.enter_context(tc.tile_pool(name="sxnp", bufs=2))
                if tr_mode == "sign8" else None)

        tr_dt = mm_dt if tr_mode == "sign8" else FP32
        ident = const.tile([P, P], tr_dt)
        make_identity(nc, ident)
        sbias = const.tile([P, 1], FP32)   # tiny bias so sign(0+eps) = +1
        nc.vector.memset(sbias, SIGN_BIAS)

        sk = skp.tile([P, KT, F], mm_dt)          # Sk signs, [d_lo, d_hi, f]
        bkb = const.tile([P, F], FP32)            # 0.5*(bk+eps), bcast on parts
        biasb = const.tile([P, F], FP32) if has_bias else None
        halfb = const.tile([P, 1], FP32)
        nc.vector.memset(halfb, 0.5)

        mxk = const.tile([P, F], FP32)
        mnk = const.tile([P, F], FP32)

        def emit_k1(kload, kt):
            # one kernel stream: DMA on the (idle) gpsimd queue so x loads on
            # sync are never blocked; ACT computes Sk signs (+-1, 0 -> +1),
            # DVE tracks running max/min for the per-feature bounds.
            ktile = kload.tile([P, F], FP32, tag="kl", name="ktile")
            nc.gpsimd.dma_start(ktile, k_d[kt * P:(kt + 1) * P, :])
            nc.scalar.sign(sk[:, kt, :], ktile, bias=sbias[:])
            if kt == 0:
                nc.vector.tensor_copy(mxk, ktile)
                nc.vector.tensor_copy(mnk, ktile)
            else:
                nc.vector.tensor_tensor(mxk, mxk, ktile, op=ALU.max)
                nc.vector.tensor_tensor(mnk, mnk, ktile, op=ALU.min)

        def emit_bounds_final():
            # bk = max(mx, -mn) reduced across partitions on GPSIMD, already
            # broadcast over partitions with f on the free axis
            nc.vector.scalar_tensor_tensor(
                mxk, mnk, -1.0, mxk, op0=ALU.mult, op1=ALU.max)
            nc.gpsimd.partition_all_reduce(bkb, mxk, channels=P,
                                           reduce_op=bass_isa.ReduceOp.max)
            nc.vector.tensor_scalar(bkb, bkb, F32_EPS, 0.25,
                                    op0=ALU.add, op1=ALU.mult)
            if has_bias:
                bsrc = b_d[:]
                bbcast = bass.AP(tensor=bsrc.tensor, offset=bsrc.offset,
                                 ap=[[0, P]] + [list(pair) for pair in bsrc.ap])
                nc.sync.dma_start(biasb, bbcast)

        # ---- main loop over token blocks ----------------------------------
        # Early blocks write raw matmul counts to a DRAM scratch (their
        # epilogue is a plain ACT psum-copy with no dependency on bounds);
        # they are re-read, scaled and written to y once bkb is ready.
        kstep = 2 if double_row else 1
        pm = mybir.MatmulPerfMode.DoubleRow if double_row else None
        bx_tiles = {}
        src_tiles = {}   # per-block transpose source (x fp32 or fp8 signs)
        sxt_tiles = {}

        def emit_xload(j):
            x_t = xp.tile([P, D], FP32, tag="x", name="x_t")
            nc.sync.dma_start(x_t, x_d[j * P:(j + 1) * P, :])
            bxq = bxp.tile([P, 1], FP32, tag="bx", name="bxq")
            nc.vector.tensor_reduce(bxq, x_t, axis=AX.X, op=ALU.max,
                                    apply_absolute_value=True)
            nc.vector.tensor_scalar_add(bxq, bxq, F32_EPS)
            bx_tiles[j] = bxq
            if tr_mode == "sign8":
                sxn = sxnp.tile([P, D], mm_dt, tag="sxn", name="sxn")
                nc.scalar.sign(sxn, x_t, bias=sbias[:])
                src_tiles[j] = sxn
            else:
                src_tiles[j] = x_t
            sxt_tiles[j] = sxtp.tile([P, KT, P], mm_dt, tag="sxt", name="sxt")

        def emit_tgroup(j, g):
            # PE-transpose TG [128,128] chunks into one psum bank, then ACT
            # evacuates into sxt (computing sign for the fp32 path).
            src = src_tiles[j]
            if tr_mode == "sign8":
                # fp8 transpose mode requires output element step of 2
                tp = tpps.tile([P, TG * P, 2], tr_dt, tag="tp", name="tp")
                tpw = tp[:, :, 0]
            else:
                tp = tpps.tile([P, TG * P], tr_dt, tag="tp", name="tp")
                tpw = tp[:]
            for u in range(TG):
                kt = g * TG + u
                nc.tensor.transpose(tpw[:, u * P:(u + 1) * P],
                                    src[:, kt * P:(kt + 1) * P], ident)
            dst = sxt_tiles[j][:, g * TG:(g + 1) * TG, :]
            tpv = tpw.rearrange("p (u t) -> p u t", u=TG)
            if tr_mode == "sign8":
                nc.scalar.copy(dst, tpv)
            else:
                nc.scalar.sign(dst, tpv, bias=sbias[:])

        # merged preamble: transpose-ahead interleaved with kernel stream-1
        # slices, so PE/ACT start at once and sk tiles arrive progressively
        k1_per_j = (KT + PRE - 1) // PRE
        k1_next = 0
        with tc.tile_pool(name="kload", bufs=4) as kload:
            for j in range(PRE):
                emit_xload(j)
                for g in range(NG):
                    emit_tgroup(j, g)
                for _ in range(k1_per_j):
                    if k1_next < KT:
                        emit_k1(kload, k1_next)
                        k1_next += 1
            while k1_next < KT:
                emit_k1(kload, k1_next)
                k1_next += 1
            emit_bounds_final()

        for i in range(TB):
            if i + PRE < TB:
                emit_xload(i + PRE)
            sxt = sxt_tiles.pop(i)
            src_tiles.pop(i, None)
            mm_tiles = [mmps.tile([P, NF], FP32, tag="mm", name=f"mm{fc}")
                        for fc in range(FC)]

            for g in range(NG):
                for kt in range(g * TG, (g + 1) * TG, kstep):
                    start = kt == 0
                    stop = kt + kstep >= KT
                    for fc in range(FC):
                        if double_row:
                            nc.tensor.matmul(
                                mm_tiles[fc][:],
                                lhsT=sxt[:, kt:kt + 2, :],
                                rhs=sk[:, kt:kt + 2, fc * NF:(fc + 1) * NF],
                                start=start, stop=stop, perf_mode=pm)
                        else:
                            nc.tensor.matmul(
                                mm_tiles[fc][:],
                                lhsT=sxt[:, kt, :],
                                rhs=sk[:, kt, fc * NF:(fc + 1) * NF],
                                start=start, stop=stop)
                # keep PE's non-HAM-warm transpose stretches short by
                # interleaving the lookahead block's transpose groups
                if i + PRE < TB:
                    emit_tgroup(i + PRE, g)

            bxq = bx_tiles.pop(i)
            for fc in range(FC):
                sl = slice(fc * NF, (fc + 1) * NF)
                out_c = outp.tile([P, NF], FP32, tag="out", name="out_c")
                # y = (psum * bx[t]) * (0.25*bk[f])
                nc.vector.scalar_tensor_tensor(
                    out_c, mm_tiles[fc][:], bxq, bkb[:, sl],
                    op0=ALU.mult, op1=ALU.mult)
                if has_bias:
                    nc.vector.tensor_tensor(out_c, out_c, biasb[:, sl],
                                            op=ALU.add)
                nc.sync.dma_start(y_d[i * P:(i + 1) * P, sl], out_c)

    if not nc.is_finalized():
        nc.finalize()
    return nc


def _run(x2, ksh_list, bias_list, has_bias, mm_dt=mybir.dt.float8e4,
         double_row=True, trace=False, NF=512, tr_mode="sign8", PRE=5):
    """Compile once and run the SPMD program on all 8 cores."""
    T, D = x2.shape
    F = ksh_list[0].shape[1]
    nc = build_nc(T, D, F, mm_dt=mm_dt, double_row=double_row,
                  has_bias=has_bias, NF=NF, tr_mode=tr_mode, PRE=PRE)
    in_maps = []
    for c in range(len(ksh_list)):
        m = {"x_in": x2, "k_in": ksh_list[c]}
        if has_bias:
            m["b_in"] = bias_list[c]
        in_maps.append(m)
    res = bass_utils.run_bass_kernel_spmd(
        nc, in_maps, core_ids=list(range(len(ksh_list))), trace=trace)
    return res


def kernel(x, kernel, bias):
    x = np.ascontiguousarray(np.asarray(x, dtype=np.float32))
    k = np.ascontiguousarray(np.asarray(kernel, dtype=np.float32))
    b = np.ascontiguousarray(np.asarray(bias, dtype=np.float32))
    B, S, D = x.shape
    F = k.shape[1]
    T = B * S
    FS = F // N_CORES
    x2 = np.ascontiguousarray(x.reshape(T, D))
    has_bias = bool(np.any(b))
    ksh = [np.ascontiguousarray(k[:, c * FS:(c + 1) * FS]) for c in range(N_CORES)]
    bsh = [np.ascontiguousarray(b[c * FS:(c + 1) * FS]) for c in range(N_CORES)]
    res = _run(x2, ksh, bsh, has_bias)
    y = np.concatenate([res.results[c]["y_out"] for c in range(N_CORES)], axis=1)
    return np.ascontiguousarray(y.reshape(B, S, F)).astype(np.float32)
